# revision 3
# baseline (speedup 1.0000x reference)
"""Trainium2 Bass kernel for nn_CaptioningRNN (attention-LSTM over T=128 steps).

Sharding: tensor-parallel over the 4H gate dimension across 8 NeuronCores.
Core j owns H-slice j (128 h-rows) of each of the 4 gates, so the per-step
LSTM cell state (c, h) for that slice lives entirely on core j.

Key structure (vs. the two-AllGather baseline):
  - ONE AllGather per step: hT slice (bf16) | score partials (bf16).
  - The attention contribution attn_t @ Wattn is rewritten as
      sum_l softmax(scores)_l * B_l,   B_l := A_l @ Wattn[:, gate_slice] + b
    with B precomputed on the PE once (128 matmuls) before the loop.  This
    removes the per-step attn AllGather, the attn transpose, and 8 of the 20
    per-step matmul k-tiles; the weighted sum is a cheap DVE mul+reduce.
  - Softmax uses the real Exp activation (exp and tanh live in the same
    ScalarE table set, so there is no per-step table swap) with accum_out
    giving the denominator for free; 1/sum folds into the gate preactivation
    via one scalar_tensor_tensor: s_g = ag*rse + pa.
  - Score partials for the NEXT step are computed by DVE (A_slice^T * h,
    reduce over h) instead of the PE cross-product + diagonal mask chain.
  - Recv-side DMAs run on the Sync queue; send/out/x-prefetch DMAs on the
    Activation HWDGE queue, so post-collective reads are never head-of-line
    blocked.
  - Two filler matmul groups re-run the x k-tiles into a scratch PSUM during
    each AllGather window to keep the PE HAM clock at 8/8.
Host side does layout-only prep (slicing/transposes/casts) and the final
output assembly out[:, :, Hj] <- core j (bf16 -> f32).
"""
import numpy as np
import ml_dtypes

import bass_rust
import concourse.bass as bass
import concourse.mybir as mybir
from concourse import tile
from concourse.alu_op_type import AluOpType
from concourse.bass_utils import run_bass_kernel_spmd

BF16 = ml_dtypes.bfloat16
F32 = mybir.dt.float32
BF = mybir.dt.bfloat16
AF = mybir.ActivationFunctionType
AX = mybir.AxisListType

N, T, D, H, L, R = 64, 128, 512, 1024, 16, 8
HS, GS = H // R, 4 * H // R  # 128, 512
SCALE = 1.0 / np.sqrt(H)


def _split_waits(nc, cap=1):
    """Walrus here rejects >cap sync waits per instruction; hoist extras
    onto preceding same-engine NOPs."""
    ctr = 0
    for fn in nc.m.functions:
        for bb in fn.blocks:
            out, changed = [], False
            for ins in bb.instructions:
                si = ins.sync_info
                if si is not None and si.on_wait and len(si.on_wait) > cap:
                    waits = list(si.on_wait)
                    extra, keep = waits[:-cap], waits[-cap:]
                    for i in range(0, len(extra), cap):
                        out.append(bass_rust.InstNoOp(
                            name=f"zz_waitsplit_{ctr}", engine=ins.engine,
                            sync_info=bass_rust.SyncInfo(
                                on_wait=extra[i:i + cap], on_update=[])))
                        ctr += 1
                    ins.sync_info = bass_rust.SyncInfo(
                        on_wait=keep, on_update=list(si.on_update or []))
                    changed = True
                out.append(ins)
            if changed:
                bb.instructions = out
    return ctr


def _prep_inputs(x, A, Wx, Wh, Wattn, b):
    x = np.asarray(x, np.float32)
    A_flat = np.asarray(A, np.float32).reshape(N, H, L)
    Wx = np.asarray(Wx, np.float32)
    Wh = np.asarray(Wh, np.float32)
    Wattn = np.asarray(Wattn, np.float32)
    b = np.asarray(b, np.float32)

    h0 = A_flat.mean(axis=2).astype(np.float32)
    scores0 = (np.einsum('nh,nhl->nl', h0, A_flat) * SCALE).astype(np.float32)
    xT = np.ascontiguousarray(
        x.transpose(1, 2, 0).reshape(T, 4, 128, N)).astype(BF16)
    h0T = np.ascontiguousarray(
        h0.T.reshape(8, 128, N).transpose(1, 0, 2)).astype(BF16)
    # A^T for the B precompute: asTf[p, k, l, n] = A_flat[n, 128k+p, l]
    asTf = np.ascontiguousarray(
        A_flat.transpose(1, 2, 0).reshape(8, 128, L, N).transpose(1, 0, 2, 3)
    ).astype(BF16)

    in_maps = []
    for j in range(R):
        cols = np.array([g * H + j * HS + i for g in range(4) for i in range(HS)])
        hsl = slice(j * HS, (j + 1) * HS)
        in_maps.append({
            "xT": xT,
            "whj": np.ascontiguousarray(
                Wh[:, cols].reshape(8, 128, GS)).astype(BF16),
            "waj": np.ascontiguousarray(
                Wattn[:, cols].reshape(8, 128, GS)).astype(BF16),
            "wxj": np.ascontiguousarray(
                Wx[:, cols].reshape(4, 128, GS)).astype(BF16),
            "brep": np.tile(b[cols], (N, 1)).astype(np.float32),
            "asTf": asTf,
            "aln": np.ascontiguousarray(
                (A_flat[:, hsl, :] * SCALE).transpose(0, 2, 1)).astype(BF16),
            "eyes": np.eye(N, dtype=np.float32).astype(BF16),
            "h0T": h0T,
            "c0": np.ascontiguousarray(h0[:, hsl]),
            "s0": scores0,
        })
    return in_maps


def _build():
    nc = bass.Bass("TRN2", target_bir_lowering=False, debug=False, num_devices=R)
    rg = [list(range(R))]

    xT_d = nc.dram_tensor("xT", [T, 4, 128, N], BF, kind="ExternalInput")
    whj_d = nc.dram_tensor("whj", [8, 128, GS], BF, kind="ExternalInput")
    waj_d = nc.dram_tensor("waj", [8, 128, GS], BF, kind="ExternalInput")
    wxj_d = nc.dram_tensor("wxj", [4, 128, GS], BF, kind="ExternalInput")
    brep_d = nc.dram_tensor("brep", [N, GS], F32, kind="ExternalInput")
    asTf_d = nc.dram_tensor("asTf", [128, 8, L, N], BF, kind="ExternalInput")
    aln_d = nc.dram_tensor("aln", [N, L, HS], BF, kind="ExternalInput")
    eyes_d = nc.dram_tensor("eyes", [N, N], BF, kind="ExternalInput")
    h0T_d = nc.dram_tensor("h0T", [128, 8, N], BF, kind="ExternalInput")
    c0_d = nc.dram_tensor("c0", [N, HS], F32, kind="ExternalInput")
    s0_d = nc.dram_tensor("s0", [N, L], F32, kind="ExternalInput")
    out_d = nc.dram_tensor("out", [N, T, HS], BF, kind="ExternalOutput")

    with tile.TileContext(nc) as tc:
        with tc.tile_pool(name="const", bufs=1) as cp, \
             tc.tile_pool(name="state", bufs=1) as st, \
             tc.tile_pool(name="dram", bufs=2, space="DRAM") as dp:

            whj = cp.tile([128, 8, GS], BF, name="whj")
            wxj = cp.tile([128, 4, GS], BF, name="wxj")
            brep = cp.tile([N, GS], F32, name="brep")
            aln = cp.tile([N, L, HS], BF, name="aln")
            eyes = cp.tile([N, N], BF, name="eyes")
            B = cp.tile([N, GS, L], BF, name="B")
            nc.sync.dma_start(out=whj[:, :, :], in_=whj_d.rearrange("k p g -> p k g"))
            nc.sync.dma_start(out=wxj[:, :, :], in_=wxj_d.rearrange("k p g -> p k g"))
            nc.sync.dma_start(out=brep[:, :], in_=brep_d[:, :])
            nc.sync.dma_start(out=aln[:, :, :], in_=aln_d[:, :, :])
            nc.sync.dma_start(out=eyes[:, :], in_=eyes_d[:, :])

            c = st.tile([N, HS], F32, name="c")
            nc.sync.dma_start(out=c[:, :], in_=c0_d[:, :])

            # ---- precompute B[:, :, l] = A_l @ Wattn_slice + b ----
            with tc.tile_pool(name="pre", bufs=1) as pp, \
                 tc.tile_pool(name="ps_b", bufs=2, space="PSUM") as ps_b:
                asTf = pp.tile([128, 8, L, N], BF, name="asTf")
                waj = pp.tile([128, 8, GS], BF, name="waj")
                nc.sync.dma_start(out=asTf[:, :, :, :], in_=asTf_d[:, :, :, :])
                nc.sync.dma_start(out=waj[:, :, :], in_=waj_d.rearrange("k p g -> p k g"))
                for l in range(L):
                    pb = ps_b.tile([N, GS], F32, name="pb", tag="pb")
                    for k in range(8):
                        nc.tensor.matmul(pb[:, :], asTf[:, k, l, :], waj[:, k, :],
                                         start=(k == 0), stop=(k == 7))
                    nc.vector.tensor_add(out=B[:, :, l], in0=pb[:, :],
                                         in1=brep[:, :])

            with tc.tile_pool(name="wk", bufs=2) as wk, \
                 tc.tile_pool(name="ps_a", bufs=2, space="PSUM") as ps_a, \
                 tc.tile_pool(name="ps_t", bufs=2, space="PSUM") as ps_t, \
                 tc.tile_pool(name="ps_d", bufs=1, space="PSUM") as ps_d:

                hkt = wk.tile([128, 8, N], BF, name="hkt0", tag="hkt")
                nc.sync.dma_start(out=hkt[:, :, :], in_=h0T_d[:, :, :])
                scores = wk.tile([N, L], F32, name="scores0", tag="scores")
                nc.sync.dma_start(out=scores[:, :], in_=s0_d[:, :])
                xtile = wk.tile([128, 4, N], BF, name="xt0", tag="xtile")
                nc.scalar.dma_start(out=xtile[:, :, :],
                                    in_=xT_d[0].rearrange("k p n -> p k n"))
                sparts = None

                for t in range(T):
                    # gate preactivation accumulation: x part first (these run
                    # during the previous step's AllGather window)
                    pa = ps_a.tile([N, GS], F32, name="pa", tag="pa")
                    for kt in range(4):
                        nc.tensor.matmul(pa[:, :], xtile[:, kt, :], wxj[:, kt, :],
                                         start=(kt == 0), stop=False)
                    # filler matmuls: keep the PE HAM clock warm through the
                    # AllGather window (results never used)
                    pd = ps_d.tile([N, GS], F32, name="pd", tag="pd")
                    for rep in range(2):
                        for kt in range(4):
                            nc.tensor.matmul(pd[:, :], xtile[:, kt, :],
                                             wxj[:, kt, :],
                                             start=(rep == 0 and kt == 0),
                                             stop=(rep == 1 and kt == 3))

                    if t > 0:
                        scores = wk.tile([N, L], F32, name="scores", tag="scores")
                        nc.vector.reduce_sum(
                            out=scores[:, :],
                            in_=sparts.rearrange("n r l -> n l r"), axis=AX.X)
                    e = wk.tile([N, L], F32, name="e", tag="e")
                    se = wk.tile([N, 1], F32, name="se", tag="se")
                    nc.scalar.activation(e[:, :], scores[:, :], AF.Exp,
                                         accum_out=se[:, :])
                    rse = wk.tile([N, 1], F32, name="rse", tag="rse")
                    nc.vector.reciprocal(out=rse[:, :], in_=se[:, :])
                    e_bf = wk.tile([N, L], BF, name="e_bf", tag="e_bf")
                    nc.vector.tensor_copy(out=e_bf[:, :], in_=e[:, :])
                    tmp = wk.tile([N, GS, L], BF, name="tmp", tag="tmp")
                    nc.vector.tensor_tensor(
                        out=tmp[:, :, :], in0=B[:, :, :],
                        in1=e_bf[:, None, :].broadcast_to((N, GS, L)),
                        op=AluOpType.mult)
                    ag = wk.tile([N, GS], F32, name="ag", tag="ag")
                    nc.vector.reduce_sum(out=ag[:, :], in_=tmp[:, :, :], axis=AX.X)

                    # h part of the gate matmul (waits on the recv DMAs)
                    for r in range(8):
                        nc.tensor.matmul(pa[:, :], hkt[:, r, :], whj[:, r, :],
                                         start=False, stop=(r == 7))

                    # s_g = ag * (1/sum e) + pa
                    s_g = wk.tile([N, GS], F32, name="s_g", tag="s_g")
                    nc.vector.scalar_tensor_tensor(
                        out=s_g[:, :], in0=ag[:, :], scalar=rse[:, 0:1],
                        in1=pa[:, :], op0=AluOpType.mult, op1=AluOpType.add)

                    th3 = wk.tile([N, 3 * HS], F32, name="th3", tag="th3")
                    nc.scalar.activation(th3[:, :], s_g[:, 0:3 * HS], AF.Tanh,
                                         scale=0.5)
                    gt = wk.tile([N, HS], F32, name="gt", tag="gt")
                    nc.scalar.activation(gt[:, :], s_g[:, 3 * HS:4 * HS], AF.Tanh)
                    sig = wk.tile([N, 3 * HS], F32, name="sig", tag="sig")
                    nc.vector.tensor_scalar(out=sig[:, :], in0=th3[:, :],
                                            scalar1=1.0, scalar2=0.5,
                                            op0=AluOpType.add, op1=AluOpType.mult)
                    t1 = wk.tile([N, HS], F32, name="t1", tag="t1")
                    nc.vector.tensor_mul(out=t1[:, :], in0=sig[:, 0:HS],
                                         in1=gt[:, :])
                    nc.vector.tensor_mul(out=c[:, :], in0=sig[:, HS:2 * HS],
                                         in1=c[:, :])
                    nc.vector.tensor_add(out=c[:, :], in0=c[:, :], in1=t1[:, :])
                    tanc = wk.tile([N, HS], F32, name="tanc", tag="tanc")
                    nc.scalar.activation(tanc[:, :], c[:, :], AF.Tanh)
                    h_bf = wk.tile([N, HS], BF, name="h_bf", tag="h_bf")
                    nc.vector.tensor_mul(out=h_bf[:, :], in0=sig[:, 2 * HS:3 * HS],
                                         in1=tanc[:, :])
                    nc.scalar.dma_start(out=out_d[:, t, :], in_=h_bf[:, :])
                    if t == T - 1:
                        break

                    # hT for the AllGather payload
                    pt = ps_t.tile([128, N], BF, name="pt", tag="pt")
                    nc.tensor.transpose(pt[:, :], h_bf[:, :], eyes[:, :])
                    hT_bf = wk.tile([128, N], BF, name="hT_bf", tag="hT_bf")
                    nc.vector.tensor_copy(out=hT_bf[:, :], in_=pt[:, :])
                    sendA = dp.tile([9216], BF, name="sendA", tag="sendA")
                    nc.scalar.dma_start(
                        out=sendA[0:8192].rearrange("(p n) -> p n", p=128),
                        in_=hT_bf[:, :])
                    # score partial for the next step (DVE, overlaps the send)
                    spt = wk.tile([N, L, HS], BF, name="spt", tag="spt")
                    nc.vector.tensor_tensor(
                        out=spt[:, :, :], in0=aln[:, :, :],
                        in1=h_bf[:, None, :].broadcast_to((N, L, HS)),
                        op=AluOpType.mult)
                    spart = wk.tile([N, L], BF, name="spart", tag="spart")
                    with nc.allow_low_precision(reason="bf16 score partials"):
                        nc.vector.reduce_sum(out=spart[:, :], in_=spt[:, :, :],
                                             axis=AX.X)
                    nc.scalar.dma_start(
                        out=sendA[8192:9216].rearrange("(n l) -> n l", n=N),
                        in_=spart[:, :])

                    recvA = dp.tile([R, 9216], BF, name="recvA", tag="recvA",
                                    addr_space="Shared")
                    nc.gpsimd.collective_compute(
                        "AllGather", AluOpType.bypass, replica_groups=rg,
                        ins=[sendA[:].opt()], outs=[recvA[:, :].opt()])

                    # recv-side (Sync queue): score partials first, then the
                    # h k-tiles in two halves so matmuls can chase the DMAs
                    sparts = wk.tile([N, 8, L], BF, name="sparts", tag="sparts")
                    nc.sync.dma_start(
                        out=sparts[:, :, :],
                        in_=recvA[:, 8192:9216].rearrange("r (n l) -> n r l", n=N))
                    hkt = wk.tile([128, 8, N], BF, name="hkt", tag="hkt")
                    nc.sync.dma_start(
                        out=hkt[:, 0:4, :],
                        in_=recvA[0:4, 0:8192].rearrange("r (p n) -> p r n", p=128))
                    nc.sync.dma_start(
                        out=hkt[:, 4:8, :],
                        in_=recvA[4:8, 0:8192].rearrange("r (p n) -> p r n", p=128))

                    xtile = wk.tile([128, 4, N], BF, name="xt", tag="xtile")
                    nc.scalar.dma_start(out=xtile[:, :, :],
                                        in_=xT_d[t + 1].rearrange("k p n -> p k n"))

                # consume the filler PSUM once so the tile is read
                junk = wk.tile([N, 1], F32, name="junk", tag="junk")
                nc.vector.reduce_max(out=junk[:, :], in_=pd[:, 0:1], axis=AX.X)

    _split_waits(nc, cap=1)
    return nc


_NC_CACHE = None


def kernel(**inputs) -> np.ndarray:
    global _NC_CACHE
    in_maps = _prep_inputs(**inputs)
    if _NC_CACHE is None:
        _NC_CACHE = _build()
    res = run_bass_kernel_spmd(_NC_CACHE, in_maps, core_ids=list(range(R)))
    out = np.zeros((N, T, H), dtype=np.float32)
    for j, r in enumerate(res.results):
        out[:, :, j * HS:(j + 1) * HS] = np.asarray(r["out"]).astype(
            np.float32).reshape(N, T, HS)
    return out


# revision 7
# speedup vs baseline: 1.1707x; 1.1707x over previous
"""Trainium2 Bass kernel for nn_CaptioningRNN (attention-LSTM over T=128 steps).

Sharding: tensor-parallel over the 4H gate dimension across 8 NeuronCores.
Core j owns H-slice j (128 h-rows) of each of the 4 gates, so the per-step
LSTM cell state (c, h) for that slice lives entirely on core j.

Key structure (vs. the two-AllGather baseline):
  - ONE AllGather per step: hT slice (bf16) | score partials (bf16).
  - attn_t @ Wattn is rewritten as sum_l w_l * B_l with
      B_l := A_l @ Wattn[:, gate_slice] + b
    precomputed on the PE before the loop.  The weighted sum itself is done
    ON THE PE as 8 accumulating matmuls into the gate PSUM:
      pa += estack_lh^T @ Bstack_lh,
      estack[64a+n', lh, n] = w[n', 8a+lh] * delta(n', n)
    (one DVE multiply against a constant replicated-eye mask builds estack;
    the delta makes the partition indexing symmetric so no cross-partition
    broadcast is needed).  This removes the attn AllGather, the attn
    transpose, and the slow DVE l-reduction.
  - Softmax uses the real Exp activation (exp and tanh share a ScalarE table
    set -> no per-step table swaps).  Scores/softmax/score-partials all run
    on 128-partition layouts (l split as l = 8a + lh across partition
    halves) for full DVE width.
  - Gate activations read the PSUM directly (the attn matmuls complete the
    accumulation, bias is folded into B).
  - Recv-side DMAs on the Sync queue; send/out/x-prefetch DMAs on the
    Activation HWDGE queue -> post-collective reads are not head-of-line
    blocked.
  - Filler matmuls re-run the x k-tiles into a scratch PSUM during the
    AllGather window to keep the PE HAM clock warm.
Host side does layout-only prep (slicing/transposes/casts) and the final
output assembly out[:, :, Hj] <- core j (bf16 -> f32).
"""
import numpy as np
import ml_dtypes

import bass_rust
import concourse.bass as bass
import concourse.mybir as mybir
from concourse import tile
from concourse.alu_op_type import AluOpType
from concourse.bass_utils import run_bass_kernel_spmd

BF16 = ml_dtypes.bfloat16
F32 = mybir.dt.float32
BF = mybir.dt.bfloat16
AF = mybir.ActivationFunctionType
AX = mybir.AxisListType

N, T, D, H, L, R = 64, 128, 512, 1024, 16, 8
HS, GS = H // R, 4 * H // R  # 128, 512
LH = L // 2  # 8; l = 8a + lh with a = partition half
SCALE = 1.0 / np.sqrt(H)


def _split_waits(nc, cap=1):
    """Walrus here rejects >cap sync waits per instruction; hoist extras
    onto preceding same-engine NOPs."""
    ctr = 0
    for fn in nc.m.functions:
        for bb in fn.blocks:
            out, changed = [], False
            for ins in bb.instructions:
                si = ins.sync_info
                if si is not None and si.on_wait and len(si.on_wait) > cap:
                    waits = list(si.on_wait)
                    extra, keep = waits[:-cap], waits[-cap:]
                    for i in range(0, len(extra), cap):
                        out.append(bass_rust.InstNoOp(
                            name=f"zz_waitsplit_{ctr}", engine=ins.engine,
                            sync_info=bass_rust.SyncInfo(
                                on_wait=extra[i:i + cap], on_update=[])))
                        ctr += 1
                    ins.sync_info = bass_rust.SyncInfo(
                        on_wait=keep, on_update=list(si.on_update or []))
                    changed = True
                out.append(ins)
            if changed:
                bb.instructions = out
    return ctr


def _prep_inputs(x, A, Wx, Wh, Wattn, b):
    x = np.asarray(x, np.float32)
    A_flat = np.asarray(A, np.float32).reshape(N, H, L)
    Wx = np.asarray(Wx, np.float32)
    Wh = np.asarray(Wh, np.float32)
    Wattn = np.asarray(Wattn, np.float32)
    b = np.asarray(b, np.float32)

    h0 = A_flat.mean(axis=2).astype(np.float32)
    scores0 = (np.einsum('nh,nhl->nl', h0, A_flat) * SCALE).astype(np.float32)
    xT = np.ascontiguousarray(
        x.transpose(1, 2, 0).reshape(T, 4, 128, N)).astype(BF16)
    h0T = np.ascontiguousarray(
        h0.T.reshape(8, 128, N).transpose(1, 0, 2)).astype(BF16)
    # A^T for the B precompute: asTf[p, k, l, n] = A_flat[n, 128k+p, l]
    asTf = np.ascontiguousarray(
        A_flat.transpose(1, 2, 0).reshape(8, 128, L, N).transpose(1, 0, 2, 3)
    ).astype(BF16)
    # replicated-eye mask for estack: dmask[64a+n', lh, n] = delta(n', n)
    dmask = np.ascontiguousarray(
        np.broadcast_to(np.eye(N, dtype=np.float32)[:, None, :], (N, LH, N))
    )
    dmask = np.concatenate([dmask, dmask], axis=0).astype(BF16)
    # full duplicated initial scores: s0_4[64a+n, l] = scores0[n, l]
    s0_4 = np.concatenate([scores0, scores0], axis=0)

    in_maps = []
    for j in range(R):
        cols = np.array([g * H + j * HS + i for g in range(4) for i in range(HS)])
        hsl = slice(j * HS, (j + 1) * HS)
        # aln2[64a+n, lh, hh] = A_flat[n, j*128+hh, 8a+lh] * SCALE
        alj = (A_flat[:, hsl, :] * SCALE).transpose(0, 2, 1)  # [n, l, hh]
        aln2 = np.concatenate([alj[:, 0:LH, :], alj[:, LH:L, :]], axis=0)
        in_maps.append({
            "xT": xT,
            "whj": np.ascontiguousarray(
                Wh[:, cols].reshape(8, 128, GS)).astype(BF16),
            "waj": np.ascontiguousarray(
                Wattn[:, cols].reshape(8, 128, GS)).astype(BF16),
            "wxj": np.ascontiguousarray(
                Wx[:, cols].reshape(4, 128, GS)).astype(BF16),
            "brep2": np.tile(b[cols], (128, 1)).astype(np.float32),
            "asTf": asTf,
            "aln2": np.ascontiguousarray(aln2).astype(BF16),
            "dmask": dmask,
            "eyes": np.eye(N, dtype=np.float32).astype(BF16),
            "h0T": h0T,
            "c0": np.ascontiguousarray(h0[:, hsl]),
            "s0": np.ascontiguousarray(s0_4),
        })
    return in_maps


def _build():
    nc = bass.Bass("TRN2", target_bir_lowering=False, debug=False, num_devices=R)
    rg = [list(range(R))]

    xT_d = nc.dram_tensor("xT", [T, 4, 128, N], BF, kind="ExternalInput")
    whj_d = nc.dram_tensor("whj", [8, 128, GS], BF, kind="ExternalInput")
    waj_d = nc.dram_tensor("waj", [8, 128, GS], BF, kind="ExternalInput")
    wxj_d = nc.dram_tensor("wxj", [4, 128, GS], BF, kind="ExternalInput")
    brep2_d = nc.dram_tensor("brep2", [128, GS], F32, kind="ExternalInput")
    asTf_d = nc.dram_tensor("asTf", [128, 8, L, N], BF, kind="ExternalInput")
    aln2_d = nc.dram_tensor("aln2", [128, LH, HS], BF, kind="ExternalInput")
    dmask_d = nc.dram_tensor("dmask", [128, LH, N], BF, kind="ExternalInput")
    eyes_d = nc.dram_tensor("eyes", [N, N], BF, kind="ExternalInput")
    h0T_d = nc.dram_tensor("h0T", [128, 8, N], BF, kind="ExternalInput")
    c0_d = nc.dram_tensor("c0", [N, HS], F32, kind="ExternalInput")
    s0_d = nc.dram_tensor("s0", [128, L], F32, kind="ExternalInput")
    out_d = nc.dram_tensor("out", [N, T, HS], BF, kind="ExternalOutput")

    SEND = 8192 + 128 * LH  # hT | spart2

    with tile.TileContext(nc) as tc:
        with tc.tile_pool(name="const", bufs=1) as cp, \
             tc.tile_pool(name="state", bufs=1) as st, \
             tc.tile_pool(name="dram", bufs=2, space="DRAM") as dp:

            whj = cp.tile([128, 8, GS], BF, name="whj")
            wxj = cp.tile([128, 4, GS], BF, name="wxj")
            brep2 = cp.tile([128, GS], F32, name="brep2")
            aln2 = cp.tile([128, LH, HS], BF, name="aln2")
            dmask = cp.tile([128, LH, N], BF, name="dmask")
            eyes = cp.tile([N, N], BF, name="eyes")
            Bst = cp.tile([128, LH, GS], BF, name="Bst")
            nc.sync.dma_start(out=whj[:, :, :], in_=whj_d.rearrange("k p g -> p k g"))
            nc.sync.dma_start(out=wxj[:, :, :], in_=wxj_d.rearrange("k p g -> p k g"))
            nc.sync.dma_start(out=brep2[:, :], in_=brep2_d[:, :])
            nc.sync.dma_start(out=aln2[:, :, :], in_=aln2_d[:, :, :])
            nc.sync.dma_start(out=dmask[:, :, :], in_=dmask_d[:, :, :])
            nc.sync.dma_start(out=eyes[:, :], in_=eyes_d[:, :])

            c = st.tile([N, HS], F32, name="c")
            nc.sync.dma_start(out=c[:, :], in_=c0_d[:, :])

            # ---- precompute Bstack[64a+n', lh, :] = A_{8a+lh} @ Wattn_j + b
            with tc.tile_pool(name="pre", bufs=1) as pp, \
                 tc.tile_pool(name="ps_b", bufs=2, space="PSUM") as ps_b:
                asTf = pp.tile([128, 8, L, N], BF, name="asTf")
                waj = pp.tile([128, 8, GS], BF, name="waj")
                nc.sync.dma_start(out=asTf[:, :, :, :], in_=asTf_d[:, :, :, :])
                nc.sync.dma_start(out=waj[:, :, :], in_=waj_d.rearrange("k p g -> p k g"))
                for lh in range(LH):
                    pb = ps_b.tile([128, GS], F32, name="pb", tag="pb")
                    for a in range(2):
                        l = 8 * a + lh
                        for k in range(8):
                            nc.tensor.matmul(pb[64 * a:64 * a + 64, :],
                                             asTf[:, k, l, :], waj[:, k, :],
                                             start=(k == 0), stop=(k == 7))
                    nc.vector.tensor_add(out=Bst[:, lh, :], in0=pb[:, :],
                                         in1=brep2[:, :])

            with tc.tile_pool(name="wk", bufs=2) as wk, \
                 tc.tile_pool(name="ps_a", bufs=2, space="PSUM") as ps_a, \
                 tc.tile_pool(name="ps_t", bufs=2, space="PSUM") as ps_t, \
                 tc.tile_pool(name="ps_d", bufs=1, space="PSUM") as ps_d:

                hkt = wk.tile([128, 8, N], BF, name="hkt0", tag="hkt")
                nc.sync.dma_start(out=hkt[:, :, :], in_=h0T_d[:, :, :])
                scores = wk.tile([128, L], F32, name="scores0", tag="scores")
                nc.sync.dma_start(out=scores[:, :], in_=s0_d[:, :])
                xtile = wk.tile([128, 4, N], BF, name="xt0", tag="xtile")
                nc.scalar.dma_start(out=xtile[:, :, :],
                                    in_=xT_d[0].rearrange("k p n -> p k n"))
                sparts = None

                for t in range(T):
                    # gate preactivations: x part first (runs during the
                    # previous step's AllGather window)
                    pa = ps_a.tile([N, GS], F32, name="pa", tag="pa")
                    for kt in range(4):
                        nc.tensor.matmul(pa[:, :], xtile[:, kt, :], wxj[:, kt, :],
                                         start=(kt == 0), stop=False)
                    # filler matmuls: keep the PE HAM clock warm through the
                    # AllGather window (results never used)
                    pd = ps_d.tile([N, GS], F32, name="pd", tag="pd")
                    for rep in range(2):
                        for kt in range(4):
                            nc.tensor.matmul(pd[:, :], xtile[:, kt, :],
                                             wxj[:, kt, :],
                                             start=(rep == 0 and kt == 0),
                                             stop=(rep == 1 and kt == 3))

                    if t > 0:
                        scores = wk.tile([128, L], F32, name="scores",
                                         tag="scores")
                        nc.vector.reduce_sum(
                            out=scores[:, :],
                            in_=sparts.rearrange("p r a lh -> p a lh r"),
                            axis=AX.X)
                    e = wk.tile([128, L], F32, name="e", tag="e")
                    nc.scalar.activation(e[:, :], scores[:, :], AF.Exp)
                    se = wk.tile([128, 1], F32, name="se", tag="se")
                    nc.vector.reduce_sum(out=se[:, :], in_=e[:, :], axis=AX.X)
                    rse = wk.tile([128, 1], F32, name="rse", tag="rse")
                    nc.vector.reciprocal(out=rse[:, :], in_=se[:, :])
                    wbf = wk.tile([128, L], BF, name="wbf", tag="wbf")
                    nc.vector.tensor_scalar(out=wbf[:, :], in0=e[:, :],
                                            scalar1=rse[:, 0:1], scalar2=None,
                                            op0=AluOpType.mult)
                    estack = wk.tile([128, LH, N], BF, name="estack",
                                     tag="estack")
                    nc.vector.tensor_tensor(
                        out=estack[0:64, :, :], in0=dmask[0:64, :, :],
                        in1=wbf[0:64, 0:LH][:, :, None].broadcast_to(
                            (64, LH, N)),
                        op=AluOpType.mult)
                    nc.vector.tensor_tensor(
                        out=estack[64:128, :, :], in0=dmask[64:128, :, :],
                        in1=wbf[64:128, LH:L][:, :, None].broadcast_to(
                            (64, LH, N)),
                        op=AluOpType.mult)

                    # h part, then attn part, of the gate matmul
                    for r in range(8):
                        nc.tensor.matmul(pa[:, :], hkt[:, r, :], whj[:, r, :],
                                         start=False, stop=False)
                    for lh in range(LH):
                        nc.tensor.matmul(pa[:, :], estack[:, lh, :],
                                         Bst[:, lh, :],
                                         start=False, stop=(lh == LH - 1))

                    th3 = wk.tile([N, 3 * HS], F32, name="th3", tag="th3")
                    nc.scalar.activation(th3[:, :], pa[:, 0:3 * HS], AF.Tanh,
                                         scale=0.5)
                    gt = wk.tile([N, HS], F32, name="gt", tag="gt")
                    nc.scalar.activation(gt[:, :], pa[:, 3 * HS:4 * HS], AF.Tanh)
                    sig = wk.tile([N, 3 * HS], F32, name="sig", tag="sig")
                    nc.vector.tensor_scalar(out=sig[:, :], in0=th3[:, :],
                                            scalar1=1.0, scalar2=0.5,
                                            op0=AluOpType.add, op1=AluOpType.mult)
                    t1 = wk.tile([N, HS], F32, name="t1", tag="t1")
                    nc.vector.tensor_mul(out=t1[:, :], in0=sig[:, 0:HS],
                                         in1=gt[:, :])
                    nc.vector.tensor_mul(out=c[:, :], in0=sig[:, HS:2 * HS],
                                         in1=c[:, :])
                    nc.vector.tensor_add(out=c[:, :], in0=c[:, :], in1=t1[:, :])
                    tanc = wk.tile([N, HS], F32, name="tanc", tag="tanc")
                    nc.scalar.activation(tanc[:, :], c[:, :], AF.Tanh)
                    h_bf = wk.tile([N, HS], BF, name="h_bf", tag="h_bf")
                    nc.vector.tensor_mul(out=h_bf[:, :], in0=sig[:, 2 * HS:3 * HS],
                                         in1=tanc[:, :])
                    nc.scalar.dma_start(out=out_d[:, t, :], in_=h_bf[:, :])
                    if t == T - 1:
                        break

                    # hT for the AllGather payload
                    pt = ps_t.tile([128, N], BF, name="pt", tag="pt")
                    nc.tensor.transpose(pt[:, :], h_bf[:, :], eyes[:, :])
                    hT_bf = wk.tile([128, N], BF, name="hT_bf", tag="hT_bf")
                    nc.vector.tensor_copy(out=hT_bf[:, :], in_=pt[:, :])
                    sendA = dp.tile([SEND], BF, name="sendA", tag="sendA")
                    nc.scalar.dma_start(
                        out=sendA[0:8192].rearrange("(p n) -> p n", p=128),
                        in_=hT_bf[:, :])
                    # score partial for the next step on 128 partitions:
                    # spart2[64a+n, lh] = sum_hh aln2 * h -> needs h on both halves
                    h2 = wk.tile([128, HS], BF, name="h2", tag="h2")
                    nc.sync.dma_start(out=h2[0:64, :], in_=h_bf[:, :])
                    nc.sync.dma_start(out=h2[64:128, :], in_=h_bf[:, :])
                    spt = wk.tile([128, LH, HS], BF, name="spt", tag="spt")
                    nc.vector.tensor_tensor(
                        out=spt[:, :, :], in0=aln2[:, :, :],
                        in1=h2[:, None, :].broadcast_to((128, LH, HS)),
                        op=AluOpType.mult)
                    spart = wk.tile([128, LH], BF, name="spart", tag="spart")
                    with nc.allow_low_precision(reason="bf16 score partials"):
                        nc.vector.reduce_sum(out=spart[:, :], in_=spt[:, :, :],
                                             axis=AX.X)
                    nc.scalar.dma_start(
                        out=sendA[8192:SEND].rearrange("(p lh) -> p lh", p=128),
                        in_=spart[:, :])

                    recvA = dp.tile([R, SEND], BF, name="recvA", tag="recvA",
                                    addr_space="Shared")
                    nc.gpsimd.collective_compute(
                        "AllGather", AluOpType.bypass, replica_groups=rg,
                        ins=[sendA[:].opt()], outs=[recvA[:, :].opt()])

                    # recv (Sync queue): score partials first (both halves get
                    # the full 16 l's), then the h k-tiles in two halves
                    sparts = wk.tile([128, 8, 2, LH], BF, name="sparts",
                                     tag="sparts")
                    for a in range(2):
                        src = recvA[:, 8192 + 512 * a:8192 + 512 * (a + 1)
                                    ].rearrange("r (n lh) -> n r lh", n=N)
                        nc.sync.dma_start(out=sparts[0:64, :, a, :], in_=src)
                        nc.gpsimd.dma_start(out=sparts[64:128, :, a, :],
                                            in_=src)
                    hkt = wk.tile([128, 8, N], BF, name="hkt", tag="hkt")
                    nc.sync.dma_start(
                        out=hkt[:, 0:4, :],
                        in_=recvA[0:4, 0:8192].rearrange("r (p n) -> p r n", p=128))
                    nc.sync.dma_start(
                        out=hkt[:, 4:8, :],
                        in_=recvA[4:8, 0:8192].rearrange("r (p n) -> p r n", p=128))

                    xtile = wk.tile([128, 4, N], BF, name="xt", tag="xtile")
                    nc.scalar.dma_start(out=xtile[:, :, :],
                                        in_=xT_d[t + 1].rearrange("k p n -> p k n"))

                # consume the filler PSUM once so the tile is read
                junk = wk.tile([N, 1], F32, name="junk", tag="junk")
                nc.vector.reduce_max(out=junk[:, :], in_=pd[:, 0:1], axis=AX.X)

    _split_waits(nc, cap=1)
    return nc


_NC_CACHE = None


def kernel(**inputs) -> np.ndarray:
    global _NC_CACHE
    in_maps = _prep_inputs(**inputs)
    if _NC_CACHE is None:
        _NC_CACHE = _build()
    res = run_bass_kernel_spmd(_NC_CACHE, in_maps, core_ids=list(range(R)))
    out = np.zeros((N, T, H), dtype=np.float32)
    for j, r in enumerate(res.results):
        out[:, :, j * HS:(j + 1) * HS] = np.asarray(r["out"]).astype(
            np.float32).reshape(N, T, HS)
    return out
